# revision 2
# baseline (speedup 1.0000x reference)
"""Trainium2 Bass kernel for nn_LoRA_MoElayer (top-1 LoRA MoE, 7 experts).

Math notes (K=1 routing):
  - gates are one-hot => y = log(exp(out_e(t))) == out_e(t) exactly, so the
    exp/log combine collapses to selecting the routed expert's LoRA output.
  - Nonzero LoRA rows are stacked: A_s [392,1024], Bw_s [392,1024]
    (sum of d_e = 392 of the padded 7*128).  Then
        h^T  = A_s @ x^T                       (GEMM1, fp32r)
        hm^T = h^T * (S @ onehot^T)            (mask expand via tiny matmul)
        y    = hm^T.T @ Bw_s                   (GEMM2, fp32r)
  - Gating logits are computed in full float32 (min top1-top2 margin of this
    problem is ~1.2e-5; fp32r/bf16 would risk flipping the argmax).
  - Loss = 2 * cv^2(expert counts); counts are exact integers accumulated on
    device (free-dim reduce of onehot^T), summed + finished on host.

Sharding: data-parallel over tokens; core c gets batch row c (2048 tokens).
Weights replicated. No cross-core communication.
"""

import numpy as np
from contextlib import ExitStack

import concourse.bass as bass
import concourse.mybir as mybir
import concourse.tile as tile
from concourse import bacc
from concourse.bass_utils import run_bass_kernel_spmd

F32 = mybir.dt.float32
F32R = mybir.dt.float32r

B, N, C = 8, 2048, 1024
E = 7
LORA_DIMS = [8, 16, 32, 48, 64, 96, 128]
DS = sum(LORA_DIMS)  # 392
NCORES = 8
TLOC = (B * N) // NCORES  # 2048 tokens per core
TILE_T = 512
NTILES = TLOC // TILE_T  # 4
KJ = C // 128  # 8 contraction chunks
# row-chunks of the stacked LoRA dim (partition-dim chunks <=128)
CHUNKS = []
_r = 0
while _r < DS:
    CHUNKS.append((_r, min(128, DS - _r)))
    _r += min(128, DS - _r)


def _kernel_body(tc, xT, wg, ast, bws, st, ident, y, cnt):
    nc = tc.nc
    with ExitStack() as ctx:
        constp = ctx.enter_context(tc.tile_pool(name="const", bufs=1))
        xp = ctx.enter_context(tc.tile_pool(name="xin", bufs=2))
        sbp = ctx.enter_context(tc.tile_pool(name="sb", bufs=2))
        hmp = ctx.enter_context(tc.tile_pool(name="hm", bufs=2))
        yp = ctx.enter_context(tc.tile_pool(name="ysb", bufs=4))
        pss = ctx.enter_context(tc.tile_pool(name="ps_small", bufs=2, space="PSUM"))
        psm = ctx.enter_context(tc.tile_pool(name="ps_mask", bufs=2, space="PSUM"))
        psh = ctx.enter_context(tc.tile_pool(name="ps_h", bufs=2, space="PSUM"))
        psy = ctx.enter_context(tc.tile_pool(name="ps_y", bufs=2, space="PSUM"))

        # ---- constants (loaded once) ----
        ast_t = constp.tile([128, KJ, DS], F32R)  # A_s^T chunked by C
        for j in range(KJ):
            nc.sync.dma_start(
                ast_t[:, j, :], ast.rearrange("(j p) m -> j p m", p=128)[j]
            )
        bw_t = []
        for ci, (r0, rc) in enumerate(CHUNKS):
            t = constp.tile([rc, C], F32R, tag=f"bw{ci}")
            nc.sync.dma_start(t[:], bws[r0 : r0 + rc, :])
            bw_t.append(t)
        wg_t = constp.tile([128, KJ, E], F32)
        nc.sync.dma_start(wg_t[:], wg.rearrange("(j p) e -> p j e", p=128))
        st_t = constp.tile([E, DS], F32R)
        nc.sync.dma_start(st_t[:], st[:])
        id_t = constp.tile([128, 128], F32)
        nc.sync.dma_start(id_t[:], ident[:])
        cnt_sb = constp.tile([E, NTILES], F32)

        xT_r = xT.rearrange("(j p) t -> j p t", p=128)  # [8, 128, 2048]

        for ti in range(NTILES):
            t0 = ti * TILE_T
            # ---- load x^T tile [128, 8, 512] ----
            xt = xp.tile([128, KJ, TILE_T], F32R)
            for j in range(KJ):
                nc.sync.dma_start(xt[:, j, :], xT_r[j, :, t0 : t0 + TILE_T])

            # ---- gating: logits [tokens, E] in full f32, 4 subtiles ----
            lg = pss.tile([128, 4, E], F32, tag="ps_sm")
            for s in range(4):
                for j in range(KJ):
                    nc.tensor.matmul(
                        lg[:, s, :],
                        xt[:, j, s * 128 : (s + 1) * 128].bitcast(F32),
                        wg_t[:, j, :],
                        start=(j == 0),
                        stop=(j == KJ - 1),
                    )
            maxc = sbp.tile([128, 4], F32, tag="maxc")
            oh = sbp.tile([128, 4, E], F32, tag="oh")
            for s in range(4):
                nc.vector.reduce_max(
                    maxc[:, s : s + 1], lg[:, s, :], axis=mybir.AxisListType.X
                )
                nc.vector.tensor_scalar(
                    oh[:, s, :],
                    lg[:, s, :],
                    maxc[:, s : s + 1],
                    None,
                    op0=mybir.AluOpType.is_ge,
                )
            # one-hot^T [E, 512] via PE transpose
            ohT_ps = pss.tile([E, TILE_T], F32, tag="ps_sm")
            for s in range(4):
                nc.tensor.transpose(
                    ohT_ps[:, s * 128 : (s + 1) * 128], oh[:, s, :], id_t[:]
                )
            ohT = sbp.tile([E, TILE_T], F32R, tag="ohT")
            nc.any.tensor_copy(ohT[:], ohT_ps[:])
            # expert counts for this tile
            nc.vector.reduce_sum(
                cnt_sb[:, ti : ti + 1], ohT[:].bitcast(F32), axis=mybir.AxisListType.X
            )

            # ---- GEMM1 + mask per row-chunk ----
            hms = []
            for ci, (r0, rc) in enumerate(CHUNKS):
                mask_ps = psm.tile([rc, TILE_T], F32, tag="ps_mask")
                nc.tensor.matmul(
                    mask_ps[:],
                    st_t[:, r0 : r0 + rc],
                    ohT[:],
                    start=True,
                    stop=True,
                )
                h_ps = psh.tile([rc, TILE_T], F32, tag="ps_h")
                for j in range(KJ):
                    nc.tensor.matmul(
                        h_ps[:],
                        ast_t[:, j, r0 : r0 + rc],
                        xt[:, j, :],
                        start=(j == 0),
                        stop=(j == KJ - 1),
                    )
                hsb = sbp.tile([rc, TILE_T], F32, tag=f"hsb{ci}")
                nc.any.tensor_copy(hsb[:], h_ps[:])
                hm = hmp.tile([rc, TILE_T], F32R, tag=f"hm{ci}")
                nc.any.tensor_tensor(hm[:], hsb[:], mask_ps[:], op=mybir.AluOpType.mult)
                hms.append(hm)

            # ---- GEMM2: y[tokens, C] per 128-token subtile, 512-col halves ----
            for s in range(4):
                for nh in range(2):
                    y_ps = psy.tile([128, 512], F32, tag="ps_y")
                    for ci, (r0, rc) in enumerate(CHUNKS):
                        nc.tensor.matmul(
                            y_ps[:],
                            hms[ci][:, s * 128 : (s + 1) * 128],
                            bw_t[ci][:, nh * 512 : (nh + 1) * 512],
                            start=(ci == 0),
                            stop=(ci == len(CHUNKS) - 1),
                        )
                    ysb = yp.tile([128, 512], F32, tag="ysb")
                    nc.any.tensor_copy(ysb[:], y_ps[:])
                    nc.sync.dma_start(
                        y[t0 + s * 128 : t0 + (s + 1) * 128, nh * 512 : (nh + 1) * 512],
                        ysb[:],
                    )

        nc.sync.dma_start(cnt[:], cnt_sb[:])


_CACHE = {}


def _build():
    if "nc" in _CACHE:
        return _CACHE["nc"]
    nc = bacc.Bacc("TRN2", target_bir_lowering=False, debug=False)
    xT_d = nc.dram_tensor("xT", [C, TLOC], F32R, kind="ExternalInput").ap()
    wg_d = nc.dram_tensor("wg", [C, E], F32, kind="ExternalInput").ap()
    ast_d = nc.dram_tensor("ast", [C, DS], F32R, kind="ExternalInput").ap()
    bws_d = nc.dram_tensor("bws", [DS, C], F32R, kind="ExternalInput").ap()
    st_d = nc.dram_tensor("st", [E, DS], F32R, kind="ExternalInput").ap()
    id_d = nc.dram_tensor("ident", [128, 128], F32, kind="ExternalInput").ap()
    y_d = nc.dram_tensor("y", [TLOC, C], F32, kind="ExternalOutput").ap()
    cnt_d = nc.dram_tensor("counts", [E, NTILES], F32, kind="ExternalOutput").ap()
    with tile.TileContext(nc) as tc:
        _kernel_body(tc, xT_d, wg_d, ast_d, bws_d, st_d, id_d, y_d, cnt_d)
    nc.compile()
    _CACHE["nc"] = nc
    return nc


def make_host_inputs(x, w_gate, A, Bw):
    """Host-side prep: stacked weights + per-core transposed x shards."""
    x = np.ascontiguousarray(np.asarray(x, dtype=np.float32))
    w_gate = np.ascontiguousarray(np.asarray(w_gate, dtype=np.float32))
    A = np.asarray(A, dtype=np.float32)
    Bw = np.asarray(Bw, dtype=np.float32)
    A_s = np.concatenate([A[e, :d, :] for e, d in enumerate(LORA_DIMS)], 0)  # [392,C]
    ast = np.ascontiguousarray(A_s.T)  # [C, 392]
    bws = np.ascontiguousarray(
        np.concatenate([Bw[e][:, :d].T for e, d in enumerate(LORA_DIMS)], 0)
    )  # [392, C]
    st = np.zeros((E, DS), np.float32)
    r = 0
    for e, d in enumerate(LORA_DIMS):
        st[e, r : r + d] = 1.0
        r += d
    ident = np.eye(128, dtype=np.float32)
    xf = x.reshape(NCORES, TLOC, C)
    in_maps = []
    for c_ in range(NCORES):
        in_maps.append(
            {
                "xT": np.ascontiguousarray(xf[c_].T),
                "wg": w_gate,
                "ast": ast,
                "bws": bws,
                "st": st,
                "ident": ident,
            }
        )
    return in_maps


def _cv2_f32(v):
    v = v.astype(np.float32)
    return np.float32(np.var(v, ddof=1)) / (np.float32(np.mean(v)) ** 2 + np.float32(1e-10))


def assemble(results):
    y = np.stack([results[c]["y"] for c in range(NCORES)], 0).reshape(B, N, C)
    counts = np.zeros(E, np.float64)
    for c_ in range(NCORES):
        counts += results[c_]["counts"].sum(axis=1)
    imp = counts.astype(np.float32)
    loss = _cv2_f32(imp) + _cv2_f32(imp)
    return y, np.float32(loss)


def kernel(x, w_gate, A, Bw):
    in_maps = make_host_inputs(x, w_gate, A, Bw)
    nc = _build()
    res = run_bass_kernel_spmd(nc, in_maps, list(range(NCORES)))
    return assemble(res.results)
